# revision 39
# baseline (speedup 1.0000x reference)
"""Adaptive-softmax loss (nn_AdaptiveLoss) on 8 trn2 NeuronCores.

Strategy: tensor-parallel over the vocab dimension, 8-way. Each core owns
1/8 of the shortlist head columns and 1/8 of each tail cluster's output
rows. Per core:

  - computes cluster hidden states h_g = x @ proj_g.T (replicated, small),
  - computes its slice of every group's logits with fp8 matmuls
    (DoubleRow perf mode merges pairs of K=128 tiles for the K>=256
    groups: head, c0, c1 and the h projections),
  - exp()s the logits on the scalar engine (partial softmax denominators
    Z_g[b] accumulate on ACT for c1/c2/c3 and on DVE for head/c0),
  - gathers exp(logit) at this core's share of the targets straight out of
    SBUF (gpsimd indirect_copy; indices + per-target loss weights are
    routed host-side as part of sharding),
  - per-row target-weight sums (den, W_g) are computed host-side and
    shipped as tiny fp32 constants - they need no device work and no
    AllReduce,
  - one 24KB AllReduce combines {Zh, Zc_g, numraw}, then every core
    finishes the (cheap) log/normalize arithmetic identically.

The full [B, VOCAB] log-prob matrix is never materialized anywhere.
"""

import sys

sys.path.insert(0, "/opt/trn_rl_repo")

from contextlib import ExitStack

import ml_dtypes
import numpy as np

import concourse.bass as bass  # noqa: F401  (engine types via nc.*)
import concourse.mybir as mybir
import concourse.tile as tile
from concourse import bacc
from concourse.bass_utils import run_bass_kernel_spmd

BF16 = ml_dtypes.bfloat16
FP8 = ml_dtypes.float8_e4m3
F32 = mybir.dt.float32
BF16_DT = mybir.dt.bfloat16
FP8_DT = mybir.dt.float8e4
U16 = mybir.dt.uint16

NCORES = 8
B, T, D = 1024, 128, 1024
VOCAB, SHORT = 100000, 10000
CL_SIZES = [10000, 20000, 40000, 20000]
CL_D = [512, 256, 128, 64]
SH_SHARD = SHORT // NCORES                      # 1250
CL_SHARD = [s // NCORES for s in CL_SIZES]      # 1250 2500 5000 2500
GRP_BOUNDS = [0, 10000, 20000, 40000, 80000, 100000]
GRP_SHARD = [SH_SHARD] + CL_SHARD

# fp8 scaling: weights are stored x64 (avoids the e4m3 subnormal floor at
# sigma=0.02), h is stored x16. head psum = 64*logit, cluster psum =
# 16*64*logit; the exp() folds the rescale into its scale operand.
WSCALE = 64.0
HSCALE = 16.0

# per-core concatenated logits layout: [head | links(4) | c0 | c1 | c2 | c3]
OFF_HEAD = 0
OFF_LINK = SH_SHARD                              # 1250
OFF_CL = [1254, 2504, 5004, 10004]
GRP_OFF = [OFF_HEAD] + OFF_CL                    # per-group concat offset
CONCAT = OFF_CL[-1] + CL_SHARD[-1]               # 12504
CONCAT_PAD = 12544
# pad slots gather column 0 (always computed, finite); their wm==0 zeroes
# the contribution.
PADIDX = 0
RT = 8                                           # row tiles of 128


# ----------------------------------------------------------------------------
# device kernel builder
# ----------------------------------------------------------------------------

_CACHE: dict[int, object] = {}


def _build(S: int):
    """Build + compile the SPMD kernel for padded slot count S (multiple of 16)."""
    if S in _CACHE:
        return _CACHE[S]
    SW = S // 16

    nc = bacc.Bacc("TRN2", target_bir_lowering=False, debug=False,
                   num_devices=NCORES)

    xt_d = nc.dram_tensor("xt", [D, B], FP8_DT, kind="ExternalInput")
    projt_d = nc.dram_tensor("projt", [D, sum(CL_D)], FP8_DT, kind="ExternalInput")
    whead_d = nc.dram_tensor("wheadt", [D, 1254], FP8_DT, kind="ExternalInput")
    wout_d = [
        nc.dram_tensor(f"wout{g}t", [CL_D[g], CL_SHARD[g]], FP8_DT,
                       kind="ExternalInput")
        for g in range(4)
    ]
    tix_d = nc.dram_tensor("tgtidx", [128, RT * SW], U16, kind="ExternalInput")
    wm_d = nc.dram_tensor("wm", [128, RT, S], BF16_DT, kind="ExternalInput")
    invden_d = nc.dram_tensor("invden", [128, RT], F32, kind="ExternalInput")
    wg_d = nc.dram_tensor("wg", [128, RT * 4], F32, kind="ExternalInput")
    out_d = nc.dram_tensor("out", [1, 1], F32, kind="ExternalOutput")

    EXP = mybir.ActivationFunctionType.Exp
    LN = mybir.ActivationFunctionType.Ln
    ADD = mybir.AluOpType.add
    SUB = mybir.AluOpType.subtract
    MULT = mybir.AluOpType.mult
    AXX = mybir.AxisListType.X
    DR = mybir.MatmulPerfMode.DoubleRow

    with tile.TileContext(nc) as tc, ExitStack() as ctx:
        sb = ctx.enter_context(tc.tile_pool(name="sb", bufs=1))
        big = ctx.enter_context(tc.tile_pool(name="big", bufs=4))
        ps = ctx.enter_context(tc.tile_pool(name="ps", bufs=2, space="PSUM"))
        dr = ctx.enter_context(tc.tile_pool(name="dr", bufs=1, space="DRAM"))

        # ---- persistent SBUF tensors ----
        xt_sb = sb.tile([128, 8, B], FP8_DT)           # x.T  [d, b] k-tiled
        whead_sb = sb.tile([128, 8, 1254], FP8_DT)
        wout_sb = [
            sb.tile([CL_D[g] if CL_D[g] < 128 else 128,
                     max(1, CL_D[g] // 128), CL_SHARD[g]], FP8_DT,
                    name=f"wout{g}_sb")
            for g in range(4)
        ]
        h_sb = sb.tile([128, 8, B], FP8_DT)            # h.T x16 (c3 rows 0:64 of ht7)
        tix_sb = sb.tile([128, RT * SW], U16)
        vg3 = sb.tile([128, RT, S], BF16_DT)           # gathered exp(logit)
        wm_sb = sb.tile([128, RT, S], BF16_DT)         # (1-dp) at owned targets
        logv3 = sb.tile([128, RT, S], BF16_DT)
        invden_sb = sb.tile([128, RT], F32)
        wg_sb = sb.tile([128, RT * 4], F32)
        linkexp = sb.tile([128, RT, 4], F32)
        linkraw = sb.tile([128, RT, 4], F32)           # raw link logits (= llink)
        zscr = sb.tile([128, 2560], BF16_DT)
        zs3 = sb.tile([128, RT, 3], F32)   # Z partials per piece, c2 only
        # AR payload, 6 stats x 8 row-tiles: q: 0 Zh, 1..4 Zc_g, 5 numraw
        pay = sb.tile([128, 48], F32)
        rsb = sb.tile([128, 48], F32)
        ones_sb = sb.tile([128, 1], F32)
        out_sb = sb.tile([1, 1], F32)

        pview = pay[:, :].rearrange("p (q r) -> p q r", q=6)
        rview = rsb[:, :].rearrange("p (q r) -> p q r", q=6)
        wgv = wg_sb[:, :].rearrange("p (r g) -> p r g", g=4)

        # ---- input DMAs: two hwdge queues in parallel. sync queue carries
        # the h-critical xt+projt as single fused transfers; the scalar
        # queue carries the head/cluster weights and gather aux tensors. ----
        projt_sb = big.tile([128, 8 * sum(CL_D)], FP8_DT, tag="big")
        pj = projt_sb[:, :].rearrange("p (k c) -> p k c", k=8)
        xt_r = xt_d.ap().rearrange("(k p) b -> p k b", p=128)
        pj_r = projt_d.ap().rearrange("(k p) c -> p k c", p=128)
        wh_r = whead_d.ap().rearrange("(k p) c -> p k c", p=128)
        # xt/projt split per k-pair so the h matmuls overlap the DMA-in;
        # whead split likewise so rt0's head piece starts early.
        for k in range(0, 8, 2):
            nc.sync.dma_start(out=xt_sb[:, k:k + 2, :], in_=xt_r[:, k:k + 2, :])
            nc.sync.dma_start(out=pj[:, k:k + 2, :], in_=pj_r[:, k:k + 2, :])
            nc.scalar.dma_start(out=whead_sb[:, k:k + 2, :],
                                in_=wh_r[:, k:k + 2, :])
        for g in range(4):
            prt = min(128, CL_D[g])
            nc.scalar.dma_start(
                out=wout_sb[g],
                in_=wout_d[g].ap().rearrange("(k p) c -> p k c", p=prt))
        nc.scalar.dma_start(out=tix_sb, in_=tix_d[:])
        nc.scalar.dma_start(out=wm_sb, in_=wm_d[:])
        nc.scalar.dma_start(out=invden_sb, in_=invden_d[:])
        nc.scalar.dma_start(out=wg_sb, in_=wg_d[:])

        nc.vector.memset(ones_sb[:, :], 1.0)

        # Prewarm the collectives path: a tiny dummy AllReduce early in the
        # run absorbs the first-collective ncfw setup and part of the
        # cross-core launch skew. Its result sinks into a dedicated tile
        # (nothing downstream reads it, so it serializes nothing).
        warm_src = sb.tile([1, 16], F32)
        warm_in = dr.tile([1, 16], F32)
        warm_out = dr.tile([1, 16], F32, addr_space="Shared")
        nc.vector.memset(warm_src[:, :], 1.0)
        nc.sync.dma_start(out=warm_in, in_=warm_src[:, :])
        nc.gpsimd.collective_compute(
            "AllReduce", ADD, replica_groups=[list(range(NCORES))],
            ins=[warm_in.opt()], outs=[warm_out.opt()])

        # ---- cluster hidden states h.T (all batch rows, computed locally).
        # psum = 64*h (x fp8 @ 64*proj fp8, DoubleRow over k-pairs);
        # h_sb = 16*h fp8 via ACT copy with scale 16/64.
        HT_OFF = [0, 128, 256, 384, 512, 640, 768, 896]
        HT_M = [128, 128, 128, 128, 128, 128, 128, 64]
        for bc in range(2):
            for htile in range(2):
                pst = ps.tile([128, 2048], F32, tag="ps", name=f"hps_{bc}_{htile}")
                for hl in range(4):
                    ht = htile * 4 + hl
                    M = HT_M[ht]
                    for kp in range(4):
                        nc.tensor.matmul(
                            pst[0:M, hl * 512:hl * 512 + 512],
                            pj[:, 2 * kp:2 * kp + 2, HT_OFF[ht]:HT_OFF[ht] + M],
                            xt_sb[:, 2 * kp:2 * kp + 2, bc * 512:(bc + 1) * 512],
                            start=(kp == 0), stop=(kp == 3), perf_mode=DR)
                for hl in range(4):
                    ht = htile * 4 + hl
                    M = HT_M[ht]
                    nc.vector.tensor_scalar(
                        h_sb[0:M, ht, bc * 512:(bc + 1) * 512],
                        pst[0:M, hl * 512:hl * 512 + 512],
                        HSCALE / WSCALE, 0.0, op0=MULT, op1=ADD)

        # ---- main loop: logits -> exp (+Z accumulate) -> gather, per row tile --
        # groups: (concat_off, width, n_kpairs, doublerow, lhsT source, rhs)
        def lh_head(kp, rt, two):
            return xt_sb[:, kp * two:kp * two + two, rt * 128:(rt + 1) * 128]

        def mk_lh_cl(ht0, prt):
            def f(kp, rt, two):
                if two == 1:
                    return h_sb[0:prt, ht0 + kp, rt * 128:(rt + 1) * 128]
                return h_sb[0:prt, ht0 + kp * two:ht0 + kp * two + two,
                            rt * 128:(rt + 1) * 128]
            return f

        def mk_rhs(tile_):
            def f(kp, a, w, two):
                if two == 1:
                    return tile_[:, kp, a:a + w]
                return tile_[:, kp * two:kp * two + two, a:a + w]
            return f

        GROUPS = [
            (0, 1254, 4, True, lh_head, mk_rhs(whead_sb)),
            (OFF_CL[0], 1250, 2, True, mk_lh_cl(0, 128), mk_rhs(wout_sb[0])),
            (OFF_CL[1], 2500, 1, True, mk_lh_cl(4, 128), mk_rhs(wout_sb[1])),
            (OFF_CL[2], 5000, 1, False, mk_lh_cl(6, 128), mk_rhs(wout_sb[2])),
            (OFF_CL[3], 2500, 1, False, mk_lh_cl(7, 64), mk_rhs(wout_sb[3])),
        ]

        def group_pieces(gi):
            width = GROUPS[gi][1]
            out, a, pi = [], 0, 0
            while a < width:
                w = min(2048, width - a)
                out.append((gi, pi, a, w))
                a += w
                pi += 1
            return out

        # Emission order interleaves the low-K (ACT-bound, PE-light) c2/c3
        # pieces between the K-heavy head/c0/c1 pieces.
        P_HEAD, P_C0 = group_pieces(0), group_pieces(1)
        P_C1, P_C2, P_C3 = group_pieces(2), group_pieces(3), group_pieces(4)
        PLAN = [P_C2[0], P_HEAD[0], P_C2[1], P_C0[0], P_C2[2], P_C1[0],
                P_C3[0], P_C1[1], P_C3[1]]
        # fixed Z-partial slot per c2 piece (c2 keeps ACT-accumulated Z; the
        # other groups' Z partials run on DVE where there is slack)
        ZMAP = {(3, 0): 0, (3, 1): 1, (3, 2): 2}

        for rt in range(RT):
            expb = big.tile([128, CONCAT_PAD], BF16_DT, tag="big",
                            name=f"expb_{rt}")
            for gi, pi, poff, pw in PLAN:
                goff, width, kt, isdr, lh, rh = GROUPS[gi]
                escale = 1.0 / WSCALE if gi == 0 else 1.0 / (WSCALE * HSCALE)
                pst = ps.tile([128, 2048], F32, tag="ps",
                              name=f"ps_{rt}_{gi}_{pi}")
                two = 2 if isdr else 1
                pm = DR if isdr else None
                subs = []
                a = 0
                while a < pw:
                    w = min(512, pw - a)
                    subs.append((a, w))
                    a += w
                for kp in range(kt):
                    for a, w in subs:
                        nc.tensor.matmul(
                            pst[:, a:a + w],
                            lh(kp, rt, two), rh(kp, poff + a, w, two),
                            start=(kp == 0), stop=(kp == kt - 1),
                            perf_mode=pm)
                if gi == 3:
                    # c2 pieces fold their Z partial into the exp call on ACT
                    nc.scalar.activation(
                        expb[:, goff + poff:goff + poff + pw],
                        pst[:, 0:pw], EXP, scale=escale,
                        accum_out=zs3[:, rt, ZMAP[(gi, pi)]:ZMAP[(gi, pi)] + 1])
                else:
                    nc.scalar.activation(
                        expb[:, goff + poff:goff + poff + pw],
                        pst[:, 0:pw], EXP, scale=escale)
                if gi == 0:
                    # raw link logits straight from PSUM: llink = psum/64.
                    # (exp of them is also in expb for the Zh-total term.)
                    nc.scalar.mul(linkraw[:, rt, :],
                                  pst[:, OFF_LINK:OFF_LINK + 4], 1.0 / WSCALE)
            # head/c0/c1/c3 Z partial sums on DVE (head must exclude link
            # columns; c1/c3 read the contiguous exp'd region across their
            # piece boundary in one call)
            for q, za, zw in [(0, 0, SH_SHARD),
                              (1, OFF_CL[0], CL_SHARD[0]),
                              (2, OFF_CL[1], CL_SHARD[1]),
                              (4, OFF_CL[3], CL_SHARD[3])]:
                nc.vector.tensor_scalar(
                    zscr[:, 0:zw],
                    expb[:, za:za + zw], 1.0, 0.0, op0=MULT, op1=ADD,
                    accum_out=pview[:, q, rt:rt + 1])
            # raw exp(link logits) for the final combine (DVE copy; tiny)
            nc.vector.tensor_copy(linkexp[:, rt, :],
                                  expb[:, OFF_LINK:OFF_LINK + 4])
            # gather exp(logit) at this core's targets
            nc.gpsimd.indirect_copy(
                vg3[:, rt, :], expb[:, :],
                tix_sb[:, rt * SW:(rt + 1) * SW], True)


        # combine the c2 Z piece-partials into the payload
        t8z = sb.tile([128, 8], F32)
        nc.vector.tensor_tensor(t8z[:, :], zs3[:, :, 0], zs3[:, :, 1], ADD)
        nc.vector.tensor_tensor(pview[:, 3, :], t8z[:, :], zs3[:, :, 2], ADD)

        # ---- numerator statistic (needs log of gathered exp) ----
        nc.scalar.activation(
            logv3[:, :, :].rearrange("p a b -> p (a b)"),
            vg3[:, :, :].rearrange("p a b -> p (a b)"), LN)
        tmp3 = vg3
        nc.vector.tensor_tensor(tmp3[:, :, :], logv3[:, :, :], wm_sb[:, :, :], MULT)
        nc.vector.tensor_reduce(pview[:, 5, :], tmp3[:, :, :], AXX, ADD)

        # ---- AllReduce the statistics (12KB, bf16: Z/numraw partials only
        # need ~3 significant digits against the 2e-2 tolerance) ----
        pay16 = sb.tile([128, 48], BF16_DT)
        rsb16 = sb.tile([128, 48], BF16_DT)
        cc_in = dr.tile([128, 48], BF16_DT)
        cc_out = dr.tile([128, 48], BF16_DT, addr_space="Shared")
        nc.vector.tensor_copy(pay16[:, :], pay[:, :])
        nc.sync.dma_start(out=cc_in, in_=pay16[:, :])
        nc.gpsimd.collective_compute(
            "AllReduce", ADD,
            replica_groups=[list(range(NCORES))],
            ins=[cc_in.opt()], outs=[cc_out.opt()])
        nc.sync.dma_start(out=rsb16, in_=cc_out)
        nc.vector.tensor_copy(rsb[:, :], rsb16[:, :])

        # ---- final combine (identical on every core) ----
        lsum = sb.tile([128, 8], F32)
        zf = sb.tile([128, 8], F32)
        lzh = sb.tile([128, 8], F32)
        lzc = sb.tile([128, 32], F32)
        s8 = sb.tile([128, 8], F32)
        tA = sb.tile([128, 8], F32)
        num8 = sb.tile([128, 8], F32)
        pcol = sb.tile([128, 1], F32)

        # per_sample = numraw/den + sum_g (Wg/den)(llink_g - lzc_g) - lzh
        # (invden and wg := Wg/den are host-computed constants)
        nc.vector.tensor_reduce(lsum[:, :], linkexp[:, :, :], AXX, ADD)
        nc.vector.tensor_tensor(zf[:, :], rview[:, 0, :], lsum[:, :], ADD)
        nc.scalar.activation(lzh[:, :], zf[:, :], LN)
        nc.scalar.activation(lzc[:, :], rsb[:, 8:40], LN)
        lzc3 = lzc[:, :].rearrange("p (g r) -> p g r", g=4)
        llink3 = linkraw[:, :, :]
        for g in range(4):
            nc.vector.tensor_tensor(tA[:, :], llink3[:, :, g], lzc3[:, g, :], SUB)
            if g == 0:
                nc.vector.tensor_tensor(s8[:, :], tA[:, :], wgv[:, :, g], MULT)
            else:
                nc.vector.tensor_tensor(tA[:, :], tA[:, :], wgv[:, :, g], MULT)
                nc.vector.tensor_tensor(s8[:, :], s8[:, :], tA[:, :], ADD)
        nc.vector.tensor_tensor(tA[:, :], rview[:, 5, :], invden_sb[:, :], MULT)
        nc.vector.tensor_tensor(num8[:, :], tA[:, :], s8[:, :], ADD)
        nc.vector.tensor_tensor(num8[:, :], num8[:, :], lzh[:, :], SUB)
        nc.vector.tensor_reduce(pcol[:, :], num8[:, :], AXX, ADD)
        psq = ps.tile([1, 1], F32, tag="ps")
        nc.tensor.matmul(psq[0:1, 0:1], pcol[:, 0:1], ones_sb[:, 0:1],
                         start=True, stop=True)
        nc.scalar.mul(out_sb[:, :], psq[0:1, 0:1], -1.0 / (B + 1e-5))
        nc.sync.dma_start(out=out_d[:], in_=out_sb)

    nc.compile()
    _CACHE[S] = nc
    return nc


# ----------------------------------------------------------------------------
# host-side sharding / index routing
# ----------------------------------------------------------------------------


def _shard_inputs(features, head_weight, projs, outs, discard_probs,
                  targets, target_mask):
    """Build the 8 per-core input maps. Returns (in_maps, S)."""
    xt = np.ascontiguousarray(features.T).astype(FP8)
    projt = (np.concatenate([p.T for p in projs], axis=1) * WSCALE).astype(FP8)
    linkT = head_weight[SHORT:SHORT + 4].T.astype(np.float32)

    tgt = np.asarray(targets).astype(np.int64).reshape(-1)
    msk = np.asarray(target_mask).astype(bool).reshape(-1)
    bb = np.repeat(np.arange(B, dtype=np.int64), T)

    grp = np.digitize(tgt, GRP_BOUNDS[1:-1])          # 0..4 (0 = shortlist)
    u = tgt - np.asarray(GRP_BOUNDS)[grp]
    shard = np.asarray(GRP_SHARD)[grp]
    core = u // shard
    jcat = u % shard + np.asarray(GRP_OFF)[grp]
    wval = (1.0 - discard_probs[tgt]).astype(np.float32)

    rt = bb >> 7
    gc = (bb >> 4) & 7
    part = bb & 127

    # host-side full per-row stats (identical on every core, no AllReduce):
    # 1/den[p, rt] and Wg[p, rt, g]/den
    wfull = wval * msk
    den_full = np.zeros((128, RT), np.float32)
    wg_full = np.zeros((128, RT, 4), np.float32)
    np.add.at(den_full, (part, rt), wfull)
    for g in range(4):
        sel = grp == g + 1
        np.add.at(wg_full, (part[sel], rt[sel], np.full(sel.sum(), g)),
                  wfull[sel])
    invden_full = (1.0 / den_full).astype(np.float32)
    wg_full = (wg_full * invden_full[:, :, None]).reshape(128, RT * 4)

    # padded slots per (core, rt, gc)
    key_all = ((core * RT + rt) * 8 + gc).astype(np.int64)
    valid = msk
    counts = np.bincount(key_all[valid], minlength=NCORES * RT * 8)
    S = int(counts.max())
    S = ((S + 31) // 32) * 32

    in_maps = []
    for c in range(NCORES):
        sel = valid & (core == c)
        jj = jcat[sel]
        bsel = bb[sel]
        rts = rt[sel]
        gcs = gc[sel]
        ww = wval[sel]
        po = bsel & 15
        key = rts * 8 + gcs
        order = np.argsort(key, kind="stable")
        jj, bsel, rts, gcs, po, ww = (a[order] for a in
                                      (jj, bsel, rts, gcs, po, ww))
        key = key[order]
        start_of = np.r_[0, np.flatnonzero(np.diff(key)) + 1]
        bucket_len = np.diff(np.r_[start_of, len(key)])
        slot = np.arange(len(key)) - np.repeat(start_of, bucket_len)

        tix = np.full((128, RT * (S // 16)), PADIDX, np.uint16)
        tix[16 * gcs + slot % 16, rts * (S // 16) + slot // 16] = jj.astype(np.uint16)
        wm = np.zeros((128, RT, S), np.float32)
        wm[16 * gcs + po, rts, slot] = ww
        wm = wm.astype(BF16)

        # head shard + link columns, transposed, x64 fp8
        hslice = head_weight[c * SH_SHARD:(c + 1) * SH_SHARD].T.astype(np.float32)
        wheadt = (np.concatenate([hslice, linkT], axis=1) * WSCALE).astype(FP8)
        wout_t = [
            (np.ascontiguousarray(
                outs[g][c * CL_SHARD[g]:(c + 1) * CL_SHARD[g]].T) * WSCALE
             ).astype(FP8)
            for g in range(4)
        ]
        in_maps.append({
            "xt": xt,
            "projt": projt,
            "wheadt": wheadt,
            "wout0t": wout_t[0],
            "wout1t": wout_t[1],
            "wout2t": wout_t[2],
            "wout3t": wout_t[3],
            "tgtidx": tix,
            "wm": wm,
            "invden": invden_full,
            "wg": wg_full,
        })
    return in_maps, S


def _run(features, head_weight, proj0, out0, proj1, out1, proj2, out2,
         proj3, out3, discard_probs, targets, target_mask,
         trace=False, tmpdir=None):
    features = np.asarray(features, np.float32)
    head_weight = np.asarray(head_weight, np.float32)
    projs = [np.asarray(p, np.float32) for p in (proj0, proj1, proj2, proj3)]
    outs = [np.asarray(o, np.float32) for o in (out0, out1, out2, out3)]
    discard_probs = np.asarray(discard_probs, np.float32)

    in_maps, S = _shard_inputs(features, head_weight, projs, outs,
                               discard_probs, targets, target_mask)
    nc = _build(S)
    res = run_bass_kernel_spmd(nc, in_maps, list(range(NCORES)),
                               trace=trace, tmpdir=tmpdir)
    val = np.asarray(res.results[0]["out"], np.float32).reshape(())
    return val, res


def kernel(**inputs) -> np.ndarray:
    val, _ = _run(**inputs)
    return val
